# revision 30
# baseline (speedup 1.0000x reference)
"""Multi-head self-attention forward on 8 Trainium2 NeuronCores.

Problem: x[4,2048,512] -> qkv proj (w_qkv [512,1536]) -> 8-head attention
(head_dim 64) -> out proj (w_out [512,512] + b_out) -> y[4,2048,512].

Sharding: 8 shards = (batch b in 0..3) x (head-group hg in 0..1, 4 heads each).
Core c handles b=c//2, hg=c%2. Each core computes, for its batch and its 4
heads: qkv projection (only its heads' columns), attention, and the partial
output projection restricted to its heads' rows of w_out. Host sums the two
half-projections per batch and adds the bias.

On-device layout (all "T" tensors keep the contraction dim on partitions):
  xT   [512, 2048]   x[b] transposed (host-side transpose)
  qkT  4 tiles [128, 2048]: Q01, K01, Q23, K23 (2 heads stacked per tile:
       head A on partitions 0:64, head B on 64:128)
  v_aug 16 seq-tiles [128, 4*65]: per head 64 v columns + a ones column
       (the ones column makes the oT matmul also produce the softmax
       denominator as row 64 of its output)
  sT   [k, q] scores transposed -> exp (no max subtraction: |s|~N(0,1), safe
       in fp32) -> pT
  oT   v_aug.T @ pT = [65, q]: rows 0:64 unnormalized head output (d on
       partitions), row 64 = softmax denominator
  yproj y[q,c] per head = oT_head.T @ w2_head, scaled per-partition (q) by
       1/denom via tensor_scalar, summed over the 4 heads on DVE.
"""

import numpy as np

import concourse.bass as bass
import concourse.mybir as mybir
import concourse.tile as tile
from concourse import bacc
from concourse.masks import make_identity

DIM = 512
NHEADS = 8
HD = 64
B = 4
SEQ = 2048
SCALE = HD ** -0.5

NCORES = 8
HPC = 4          # heads per core
QCH = 512        # q chunk (moving free dim)
NQC = SEQ // QCH # 4 q-chunks
KCH = 128        # k chunk (psum partition dim)
NKC = SEQ // KCH # 16 k-chunks
CCH = 128        # contraction chunk for projections
NCC = DIM // CCH # 4

F32 = mybir.dt.float32
F32R = mybir.dt.float32r

# matmul input dtype: float32r (PE rounds to reduced mantissa, 1 cycle/row at
# N>=256 vs 4 for fp32). The BIR verifier requires f32r matmul operands to be
# produced by rounding ops (ACT/DVE), so tiles feeding matmuls are declared
# f32r and filled via compute copies, never raw DMA.
MMDT = F32R


def _mm(ap):
    return ap


def build_nc():
    nc = bacc.Bacc()

    xT_d = nc.dram_tensor("xt", [DIM, SEQ], MMDT, kind="ExternalInput")
    wperm_d = nc.dram_tensor("wperm", [DIM, 4 * 128], MMDT, kind="ExternalInput")
    wv_d = nc.dram_tensor("wv", [DIM, HPC * HD], MMDT, kind="ExternalInput")
    w2_d = nc.dram_tensor("w2", [HPC * HD, DIM], MMDT, kind="ExternalInput")
    y_d = nc.dram_tensor("y", [SEQ, DIM], F32, kind="ExternalOutput")

    with tile.TileContext(nc) as tc:
        with (
            tc.tile_pool(name="const", bufs=1) as cpool,
            tc.tile_pool(name="big", bufs=1) as bigpool,
            tc.tile_pool(name="pt", bufs=3) as ptpool,
            tc.tile_pool(name="yacc", bufs=1) as yaccpool,
            tc.tile_pool(name="tmp", bufs=3) as tmppool,
            tc.tile_pool(name="small", bufs=2) as smallpool,
            tc.tile_pool(name="ps", bufs=1, space="PSUM") as ps,
        ):
            # ---- constants / inputs to SBUF ----
            # DMA lands fp32 in a staging tile; a DVE copy rounds into the
            # f32r tile the matmuls read (BIR f32r-rounding requirement).
            xTs = [cpool.tile([128, SEQ], MMDT, tag=f"xT{c}", name=f"xT{c}")
                   for c in range(NCC)]
            wps = [cpool.tile([128, 512], MMDT, tag=f"wp{c}", name=f"wp{c}")
                   for c in range(NCC)]
            wvs = [cpool.tile([128, HPC * HD], MMDT, tag=f"wv{c}", name=f"wv{c}")
                   for c in range(NCC)]
            w2s = [cpool.tile([128, DIM], MMDT, tag=f"w2{p}", name=f"w2{p}")
                   for p in range(2)]
            ident = cpool.tile([2, 2], F32, tag="ident")
            ones4 = cpool.tile([128, HPC], F32, tag="ones4")
            nc.gpsimd.memset(ones4[:], 1.0)
            ones1 = cpool.tile([1, 1], F32, tag="ones1")
            nc.gpsimd.memset(ones1[:], 1.0)

            for c in range(NCC):
                nc.sync.dma_start(xTs[c][:], xT_d[c * 128:(c + 1) * 128, :])
                nc.sync.dma_start(wps[c][:], wperm_d[c * 128:(c + 1) * 128, :])
                nc.sync.dma_start(wvs[c][:], wv_d[c * 128:(c + 1) * 128, :])
            for p in range(2):
                nc.sync.dma_start(w2s[p][:], w2_d[p * 128:(p + 1) * 128, :])
            make_identity(nc, ident[:])

            def xT_c(c):
                return xTs[c]

            # ---- persistent intermediates ----
            qkT = bigpool.tile([128, 4 * SEQ], MMDT, tag="qkT")  # Q01 K01 Q23 K23
            vaug = bigpool.tile([128, NKC * HPC * 65], MMDT, tag="vaug")
            oT = bigpool.tile([128, 2 * SEQ], MMDT, tag="oT")    # pair-packed

            yacc = yaccpool.tile([128, SEQ // 128 * DIM], F32, tag="yacc")

            def qkT_blk(m):
                return qkT[:, m * SEQ:(m + 1) * SEQ]

            def vaug_t(kc):
                # [128, HPC, 65] view of seq-tile kc
                return vaug[:, kc * HPC * 65:(kc + 1) * HPC * 65].rearrange(
                    "p (h e) -> p h e", e=65
                )

            # ---- phase 1a: qkT = wperm.T @ xT ----
            for m in range(4):
                for s in range(NQC):
                    pp = ps.tile([128, QCH], F32, tag="s", bufs=3, name="pp")
                    for c in range(NCC):
                        nc.tensor.matmul(
                            pp[:],
                            _mm(wps[c][:, m * 128:(m + 1) * 128]),
                            _mm(xT_c(c)[:, s * QCH:(s + 1) * QCH]),
                            start=(c == 0),
                            stop=(c == NCC - 1),
                            skip_group_check=True,
                        )
                    nc.vector.tensor_copy(
                        qkT_blk(m)[:, s * QCH:(s + 1) * QCH], pp[:]
                    )

            # ---- phase 1b: v = x @ wv (natural layout, + ones column) ----
            for st in range(NKC):
                pv = ps.tile([128, HPC * HD], F32, tag="s", bufs=3, name="pv")
                for c in range(NCC):
                    nc.tensor.matmul(
                        pv[:],
                        _mm(xT_c(c)[:, st * 128:(st + 1) * 128]),
                        _mm(wvs[c][:]),
                        start=(c == 0),
                        stop=(c == NCC - 1),
                        skip_group_check=True,
                    )
                vt = vaug_t(st)
                nc.vector.tensor_copy(
                    vt[:, :, 0:64], pv[:].rearrange("p (h d) -> p h d", d=HD)
                )
                nc.vector.tensor_copy(vt[:, :, 64:65],
                                      ones4[:].rearrange("p (h o) -> p h o", o=1))

            # ---- phase 2: attention + out-proj, per head-pair ----
            for p in range(2):
                Q = qkT_blk(2 * p)
                K = qkT_blk(2 * p + 1)
                for qc in range(NQC):
                    oA = ps.tile([65, QCH], F32, tag="o", bufs=2, name="oA")
                    oB = ps.tile([65, QCH], F32, tag="o", bufs=2, name="oB")
                    for kc in range(NKC):
                        sA = ps.tile([128, QCH], F32, tag="s", bufs=3, name="sA")
                        sB = ps.tile([128, QCH], F32, tag="s", bufs=3, name="sB")
                        nc.tensor.matmul(
                            sA[:],
                            _mm(K[0:64, kc * 128:(kc + 1) * 128]),
                            _mm(Q[0:64, qc * QCH:(qc + 1) * QCH]),
                            start=True, stop=True, skip_group_check=True,
                        )
                        nc.tensor.matmul(
                            sB[:],
                            _mm(K[64:128, kc * 128:(kc + 1) * 128]),
                            _mm(Q[64:128, qc * QCH:(qc + 1) * QCH]),
                            start=True, stop=True, skip_group_check=True,
                        )
                        pA = ptpool.tile([128, QCH], MMDT, tag="pA")
                        pB = ptpool.tile([128, QCH], MMDT, tag="pB")
                        nc.scalar.activation(
                            pA[:], sA[:], mybir.ActivationFunctionType.Exp,
                            scale=SCALE,
                        )
                        nc.scalar.activation(
                            pB[:], sB[:], mybir.ActivationFunctionType.Exp,
                            scale=SCALE,
                        )
                        nc.tensor.matmul(
                            oA[:],
                            _mm(vaug_t(kc)[:, 2 * p, :]),
                            _mm(pA[:]),
                            start=(kc == 0), stop=(kc == NKC - 1),
                            skip_group_check=True,
                        )
                        nc.tensor.matmul(
                            oB[:],
                            _mm(vaug_t(kc)[:, 2 * p + 1, :]),
                            _mm(pB[:]),
                            start=(kc == 0), stop=(kc == NKC - 1),
                            skip_group_check=True,
                        )
                    qs = slice(qc * QCH, (qc + 1) * QCH)
                    nc.vector.tensor_copy(oT[0:64, p * SEQ + qc * QCH:
                                             p * SEQ + (qc + 1) * QCH], oA[0:64, :])
                    nc.vector.tensor_copy(oT[64:128, p * SEQ + qc * QCH:
                                             p * SEQ + (qc + 1) * QCH], oB[0:64, :])
                    denA = smallpool.tile([1, QCH], F32, tag="denA")
                    denB = smallpool.tile([1, QCH], F32, tag="denB")
                    recA = smallpool.tile([1, QCH], F32, tag="recA")
                    recB = smallpool.tile([1, QCH], F32, tag="recB")
                    nc.vector.tensor_copy(denA[:], oA[64:65, :])
                    nc.vector.tensor_copy(denB[:], oB[64:65, :])
                    nc.vector.reciprocal(recA[:], denA[:])
                    nc.vector.reciprocal(recB[:], denB[:])

                    # transpose the 4 q-tiles' recips into per-partition layout
                    rt_ps = ps.tile([128, 2 * (QCH // 128)], F32, tag="rt",
                                    bufs=1, name="rt_ps")
                    for j in range(QCH // 128):
                        for h, rv in enumerate((recA, recB)):
                            # transpose [1,128] -> [128,1] as a K=1 matmul
                            # (is_transpose matmuls crash the HW here)
                            nc.tensor.matmul(
                                rt_ps[:, 2 * j + h:2 * j + h + 1],
                                rv[:, j * 128:(j + 1) * 128],
                                ones1[:],
                                start=True, stop=True, skip_group_check=True,
                            )
                    rt = smallpool.tile([128, 2 * (QCH // 128)], F32, tag="rt_sb")
                    nc.vector.tensor_copy(rt[:], rt_ps[:])

                    # out-projection + normalization for this q-chunk
                    for j in range(QCH // 128):
                        qt = qc * (QCH // 128) + j
                        yA = ps.tile([128, DIM], F32, tag="y", bufs=2, name="yA")
                        yB = ps.tile([128, DIM], F32, tag="y", bufs=2, name="yB")
                        oTp = oT[:, p * SEQ:(p + 1) * SEQ]
                        nc.tensor.matmul(
                            yA[:],
                            _mm(oTp[0:64, qt * 128:(qt + 1) * 128]),
                            _mm(w2s[p][0:64, :]),
                            start=True, stop=True, skip_group_check=True,
                        )
                        nc.tensor.matmul(
                            yB[:],
                            _mm(oTp[64:128, qt * 128:(qt + 1) * 128]),
                            _mm(w2s[p][64:128, :]),
                            start=True, stop=True, skip_group_check=True,
                        )
                        ya = yacc[:, qt * DIM:(qt + 1) * DIM]
                        if p == 0:
                            nc.vector.tensor_scalar_mul(ya, yA[:], rt[:, 2 * j:2 * j + 1])
                            t1 = tmppool.tile([128, DIM], F32, tag="t1")
                            nc.vector.tensor_scalar_mul(t1[:], yB[:],
                                                        rt[:, 2 * j + 1:2 * j + 2])
                            nc.vector.tensor_add(ya, ya, t1[:])
                        else:
                            t1 = tmppool.tile([128, DIM], F32, tag="t1")
                            nc.vector.tensor_scalar_mul(t1[:], yA[:],
                                                        rt[:, 2 * j:2 * j + 1])
                            nc.vector.tensor_add(ya, ya, t1[:])
                            t2 = tmppool.tile([128, DIM], F32, tag="t2")
                            nc.vector.tensor_scalar_mul(t2[:], yB[:],
                                                        rt[:, 2 * j + 1:2 * j + 2])
                            nc.vector.tensor_add(ya, ya, t2[:])
                            nc.sync.dma_start(y_d[qt * 128:(qt + 1) * 128, :], ya)

    nc.finalize()
    return nc


_NC_CACHE = {}


def get_nc():
    if "nc" not in _NC_CACHE:
        _NC_CACHE["nc"] = build_nc()
    return _NC_CACHE["nc"]


def make_core_inputs(x, w_qkv, w_out):
    """Per-core input dicts (host-side sharding)."""
    in_maps = []
    for c in range(NCORES):
        b, hg = c // 2, c % 2
        heads = [hg * HPC + i for i in range(HPC)]
        qcols = [w_qkv[:, h * HD:(h + 1) * HD] for h in heads]
        kcols = [w_qkv[:, DIM + h * HD:DIM + (h + 1) * HD] for h in heads]
        vcols = [w_qkv[:, 2 * DIM + h * HD:2 * DIM + (h + 1) * HD] for h in heads]
        wperm = np.concatenate(
            [qcols[0], qcols[1], kcols[0], kcols[1],
             qcols[2], qcols[3], kcols[2], kcols[3]], axis=1)
        wv = np.concatenate(vcols, axis=1)
        w2 = w_out[hg * HPC * HD:(hg + 1) * HPC * HD, :]
        in_maps.append({
            "xt": np.ascontiguousarray(x[b].T).astype(np.float32),
            "wperm": np.ascontiguousarray(wperm).astype(np.float32),
            "wv": np.ascontiguousarray(wv).astype(np.float32),
            "w2": np.ascontiguousarray(w2).astype(np.float32),
        })
    return in_maps


def kernel(x, w_qkv, w_out, b_out):
    from concourse.bass_utils import run_bass_kernel_spmd

    x = np.asarray(x, dtype=np.float32)
    w_qkv = np.asarray(w_qkv, dtype=np.float32)
    w_out = np.asarray(w_out, dtype=np.float32)
    b_out = np.asarray(b_out, dtype=np.float32)

    nc = get_nc()
    in_maps = make_core_inputs(x, w_qkv, w_out)
    res = run_bass_kernel_spmd(nc, in_maps, list(range(NCORES))).results

    out = np.empty((B, SEQ, DIM), dtype=np.float32)
    for b in range(B):
        out[b] = res[2 * b]["y"] + res[2 * b + 1]["y"] + b_out
    return out


# revision 31
# speedup vs baseline: 1.2252x; 1.2252x over previous
"""Multi-head self-attention forward on 8 Trainium2 NeuronCores.

Problem: x[4,2048,512] -> qkv proj (w_qkv [512,1536]) -> 8-head attention
(head_dim 64) -> out proj (w_out [512,512] + b_out) -> y[4,2048,512].

Sharding: 8 shards = (batch b in 0..3) x (head-group hg in 0..1, 4 heads each).
Core c handles b=c//2, hg=c%2. Each core computes, for its batch and its 4
heads: qkv projection (only its heads' columns), attention, and the partial
output projection restricted to its heads' rows of w_out. Host sums the two
half-projections per batch and adds the bias.

On-device layout (all "T" tensors keep the contraction dim on partitions):
  xT   [512, 2048]   x[b] transposed (host-side transpose)
  qkT  4 tiles [128, 2048]: Q01, K01, Q23, K23 (2 heads stacked per tile:
       head A on partitions 0:64, head B on 64:128)
  v_aug 16 seq-tiles [128, 4*65]: per head 64 v columns + a ones column
       (the ones column makes the oT matmul also produce the softmax
       denominator as row 64 of its output)
  sT   [k, q] scores transposed -> exp (no max subtraction: |s|~N(0,1), safe
       in fp32) -> pT
  oT   v_aug.T @ pT = [65, q]: rows 0:64 unnormalized head output (d on
       partitions), row 64 = softmax denominator
  yproj y[q,c] per head = oT_head.T @ w2_head, scaled per-partition (q) by
       1/denom via tensor_scalar, summed over the 4 heads on DVE.
"""

import numpy as np

import concourse.bass as bass
import concourse.mybir as mybir
import concourse.tile as tile
from concourse import bacc
from concourse.masks import make_identity

DIM = 512
NHEADS = 8
HD = 64
B = 4
SEQ = 2048
SCALE = HD ** -0.5

NCORES = 8
HPC = 4          # heads per core
QCH = 512        # q chunk (moving free dim)
NQC = SEQ // QCH # 4 q-chunks
KCH = 128        # k chunk (psum partition dim)
NKC = SEQ // KCH # 16 k-chunks
CCH = 128        # contraction chunk for projections
NCC = DIM // CCH # 4

F32 = mybir.dt.float32
F32R = mybir.dt.float32r

BF16 = mybir.dt.bfloat16
# matmul input dtype. bf16: 1 cycle/row guaranteed, FWL weight loads, and
# roughly half the PE power of f32r (the f32r version HAM-throttled to
# 1.2GHz for 80% of the run). f32r kept as a fallback for precision.
MMDT = BF16


def _mm(ap):
    return ap


def build_nc():
    nc = bacc.Bacc()

    xT_d = nc.dram_tensor("xt", [DIM, SEQ], MMDT, kind="ExternalInput")
    wperm_d = nc.dram_tensor("wperm", [DIM, 4 * 128], MMDT, kind="ExternalInput")
    wv_d = nc.dram_tensor("wv", [DIM, HPC * HD], MMDT, kind="ExternalInput")
    w2_d = nc.dram_tensor("w2", [HPC * HD, DIM], MMDT, kind="ExternalInput")
    y_d = nc.dram_tensor("y", [SEQ, DIM], F32, kind="ExternalOutput")

    with tile.TileContext(nc) as tc:
        with (
            tc.tile_pool(name="const", bufs=1) as cpool,
            tc.tile_pool(name="big", bufs=1) as bigpool,
            tc.tile_pool(name="pt", bufs=3) as ptpool,
            tc.tile_pool(name="yacc", bufs=1) as yaccpool,
            tc.tile_pool(name="tmp", bufs=3) as tmppool,
            tc.tile_pool(name="small", bufs=2) as smallpool,
            tc.tile_pool(name="ps", bufs=1, space="PSUM") as ps,
        ):
            # ---- constants / inputs to SBUF ----
            # DMA lands fp32 in a staging tile; a DVE copy rounds into the
            # f32r tile the matmuls read (BIR f32r-rounding requirement).
            xTs = [cpool.tile([128, SEQ], MMDT, tag=f"xT{c}", name=f"xT{c}")
                   for c in range(NCC)]
            wps = [cpool.tile([128, 512], MMDT, tag=f"wp{c}", name=f"wp{c}")
                   for c in range(NCC)]
            wvs = [cpool.tile([128, HPC * HD], MMDT, tag=f"wv{c}", name=f"wv{c}")
                   for c in range(NCC)]
            w2s = [cpool.tile([128, DIM], MMDT, tag=f"w2{p}", name=f"w2{p}")
                   for p in range(2)]
            ident = cpool.tile([2, 2], F32, tag="ident")
            ones4 = cpool.tile([128, HPC], F32, tag="ones4")
            nc.gpsimd.memset(ones4[:], 1.0)
            ones1 = cpool.tile([1, 1], F32, tag="ones1")
            nc.gpsimd.memset(ones1[:], 1.0)

            for c in range(NCC):
                nc.sync.dma_start(xTs[c][:], xT_d[c * 128:(c + 1) * 128, :])
                nc.sync.dma_start(wps[c][:], wperm_d[c * 128:(c + 1) * 128, :])
                nc.sync.dma_start(wvs[c][:], wv_d[c * 128:(c + 1) * 128, :])
            for p in range(2):
                nc.sync.dma_start(w2s[p][:], w2_d[p * 128:(p + 1) * 128, :])
            make_identity(nc, ident[:])

            def xT_c(c):
                return xTs[c]

            # ---- persistent intermediates ----
            qkT = bigpool.tile([128, 4 * SEQ], MMDT, tag="qkT")  # Q01 K01 Q23 K23
            vaug = bigpool.tile([128, NKC * HPC * 65], MMDT, tag="vaug")
            oT = bigpool.tile([128, 2 * SEQ], MMDT, tag="oT")    # pair-packed

            yacc = yaccpool.tile([128, SEQ // 128 * DIM], F32, tag="yacc")

            def qkT_blk(m):
                return qkT[:, m * SEQ:(m + 1) * SEQ]

            def vaug_t(kc):
                # [128, HPC, 65] view of seq-tile kc
                return vaug[:, kc * HPC * 65:(kc + 1) * HPC * 65].rearrange(
                    "p (h e) -> p h e", e=65
                )

            # ---- phase 1a: qkT = wperm.T @ xT ----
            for m in range(4):
                for s in range(NQC):
                    pp = ps.tile([128, QCH], F32, tag="s", bufs=3, name="pp")
                    for c in range(NCC):
                        nc.tensor.matmul(
                            pp[:],
                            _mm(wps[c][:, m * 128:(m + 1) * 128]),
                            _mm(xT_c(c)[:, s * QCH:(s + 1) * QCH]),
                            start=(c == 0),
                            stop=(c == NCC - 1),
                            skip_group_check=True,
                        )
                    nc.vector.tensor_copy(
                        qkT_blk(m)[:, s * QCH:(s + 1) * QCH], pp[:]
                    )

            # ---- phase 1b: v = x @ wv (natural layout, + ones column) ----
            for st in range(NKC):
                pv = ps.tile([128, HPC * HD], F32, tag="s", bufs=3, name="pv")
                for c in range(NCC):
                    nc.tensor.matmul(
                        pv[:],
                        _mm(xT_c(c)[:, st * 128:(st + 1) * 128]),
                        _mm(wvs[c][:]),
                        start=(c == 0),
                        stop=(c == NCC - 1),
                        skip_group_check=True,
                    )
                vt = vaug_t(st)
                nc.vector.tensor_copy(
                    vt[:, :, 0:64], pv[:].rearrange("p (h d) -> p h d", d=HD)
                )
                nc.vector.tensor_copy(vt[:, :, 64:65],
                                      ones4[:].rearrange("p (h o) -> p h o", o=1))

            # ---- phase 2: attention + out-proj, per head-pair ----
            for p in range(2):
                Q = qkT_blk(2 * p)
                K = qkT_blk(2 * p + 1)
                for qc in range(NQC):
                    oA = ps.tile([65, QCH], F32, tag="o", bufs=2, name="oA")
                    oB = ps.tile([65, QCH], F32, tag="o", bufs=2, name="oB")
                    for kc in range(NKC):
                        sA = ps.tile([128, QCH], F32, tag="s", bufs=3, name="sA")
                        sB = ps.tile([128, QCH], F32, tag="s", bufs=3, name="sB")
                        nc.tensor.matmul(
                            sA[:],
                            _mm(K[0:64, kc * 128:(kc + 1) * 128]),
                            _mm(Q[0:64, qc * QCH:(qc + 1) * QCH]),
                            start=True, stop=True, skip_group_check=True,
                        )
                        nc.tensor.matmul(
                            sB[:],
                            _mm(K[64:128, kc * 128:(kc + 1) * 128]),
                            _mm(Q[64:128, qc * QCH:(qc + 1) * QCH]),
                            start=True, stop=True, skip_group_check=True,
                        )
                        pA = ptpool.tile([128, QCH], MMDT, tag="pA")
                        pB = ptpool.tile([128, QCH], MMDT, tag="pB")
                        nc.scalar.activation(
                            pA[:], sA[:], mybir.ActivationFunctionType.Exp,
                            scale=SCALE,
                        )
                        nc.scalar.activation(
                            pB[:], sB[:], mybir.ActivationFunctionType.Exp,
                            scale=SCALE,
                        )
                        nc.tensor.matmul(
                            oA[:],
                            _mm(vaug_t(kc)[:, 2 * p, :]),
                            _mm(pA[:]),
                            start=(kc == 0), stop=(kc == NKC - 1),
                            skip_group_check=True,
                        )
                        nc.tensor.matmul(
                            oB[:],
                            _mm(vaug_t(kc)[:, 2 * p + 1, :]),
                            _mm(pB[:]),
                            start=(kc == 0), stop=(kc == NKC - 1),
                            skip_group_check=True,
                        )
                    qs = slice(qc * QCH, (qc + 1) * QCH)
                    nc.vector.tensor_copy(oT[0:64, p * SEQ + qc * QCH:
                                             p * SEQ + (qc + 1) * QCH], oA[0:64, :])
                    nc.vector.tensor_copy(oT[64:128, p * SEQ + qc * QCH:
                                             p * SEQ + (qc + 1) * QCH], oB[0:64, :])
                    denA = smallpool.tile([1, QCH], F32, tag="denA")
                    denB = smallpool.tile([1, QCH], F32, tag="denB")
                    nc.vector.tensor_copy(denA[:], oA[64:65, :])
                    nc.vector.tensor_copy(denB[:], oB[64:65, :])

                    # transpose the 4 q-tiles' recips into per-partition layout
                    rt_ps = ps.tile([128, 2 * (QCH // 128)], F32, tag="rt",
                                    bufs=1, name="rt_ps")
                    for j in range(QCH // 128):
                        for h, rv in enumerate((denA, denB)):
                            # transpose [1,128] -> [128,1] as a K=1 matmul
                            # (is_transpose matmuls crash the HW here)
                            nc.tensor.matmul(
                                rt_ps[:, 2 * j + h:2 * j + h + 1],
                                rv[:, j * 128:(j + 1) * 128],
                                ones1[:],
                                start=True, stop=True, skip_group_check=True,
                            )
                    dt_sb = smallpool.tile([128, 2 * (QCH // 128)], F32,
                                           tag="dt_sb")
                    nc.vector.tensor_copy(dt_sb[:], rt_ps[:])
                    rt = smallpool.tile([128, 2 * (QCH // 128)], F32, tag="rt_sb")
                    # reciprocal on [128, 8] (all lanes) instead of [1, 512]
                    nc.vector.reciprocal(rt[:], dt_sb[:])

                    # out-projection + normalization for this q-chunk
                    for j in range(QCH // 128):
                        qt = qc * (QCH // 128) + j
                        yA = ps.tile([128, DIM], F32, tag="y", bufs=2, name="yA")
                        yB = ps.tile([128, DIM], F32, tag="y", bufs=2, name="yB")
                        oTp = oT[:, p * SEQ:(p + 1) * SEQ]
                        nc.tensor.matmul(
                            yA[:],
                            _mm(oTp[0:64, qt * 128:(qt + 1) * 128]),
                            _mm(w2s[p][0:64, :]),
                            start=True, stop=True, skip_group_check=True,
                        )
                        nc.tensor.matmul(
                            yB[:],
                            _mm(oTp[64:128, qt * 128:(qt + 1) * 128]),
                            _mm(w2s[p][64:128, :]),
                            start=True, stop=True, skip_group_check=True,
                        )
                        ya = yacc[:, qt * DIM:(qt + 1) * DIM]
                        if p == 0:
                            nc.vector.tensor_scalar_mul(ya, yA[:], rt[:, 2 * j:2 * j + 1])
                            t1 = tmppool.tile([128, DIM], F32, tag="t1")
                            nc.vector.tensor_scalar_mul(t1[:], yB[:],
                                                        rt[:, 2 * j + 1:2 * j + 2])
                            nc.vector.tensor_add(ya, ya, t1[:])
                        else:
                            t1 = tmppool.tile([128, DIM], F32, tag="t1")
                            nc.vector.tensor_scalar_mul(t1[:], yA[:],
                                                        rt[:, 2 * j:2 * j + 1])
                            nc.vector.tensor_add(ya, ya, t1[:])
                            t2 = tmppool.tile([128, DIM], F32, tag="t2")
                            nc.vector.tensor_scalar_mul(t2[:], yB[:],
                                                        rt[:, 2 * j + 1:2 * j + 2])
                            nc.vector.tensor_add(ya, ya, t2[:])
                            nc.sync.dma_start(y_d[qt * 128:(qt + 1) * 128, :], ya)

    nc.finalize()
    return nc


_NC_CACHE = {}


def get_nc():
    if "nc" not in _NC_CACHE:
        _NC_CACHE["nc"] = build_nc()
    return _NC_CACHE["nc"]


def make_core_inputs(x, w_qkv, w_out):
    """Per-core input dicts (host-side sharding)."""
    in_maps = []
    for c in range(NCORES):
        b, hg = c // 2, c % 2
        heads = [hg * HPC + i for i in range(HPC)]
        qcols = [w_qkv[:, h * HD:(h + 1) * HD] for h in heads]
        kcols = [w_qkv[:, DIM + h * HD:DIM + (h + 1) * HD] for h in heads]
        vcols = [w_qkv[:, 2 * DIM + h * HD:2 * DIM + (h + 1) * HD] for h in heads]
        wperm = np.concatenate(
            [qcols[0], qcols[1], kcols[0], kcols[1],
             qcols[2], qcols[3], kcols[2], kcols[3]], axis=1)
        wv = np.concatenate(vcols, axis=1)
        w2 = w_out[hg * HPC * HD:(hg + 1) * HPC * HD, :]
        import ml_dtypes
        mmnp = (ml_dtypes.bfloat16 if MMDT == mybir.dt.bfloat16
                else np.float32)
        in_maps.append({
            "xt": np.ascontiguousarray(x[b].T).astype(mmnp),
            "wperm": np.ascontiguousarray(wperm).astype(mmnp),
            "wv": np.ascontiguousarray(wv).astype(mmnp),
            "w2": np.ascontiguousarray(w2).astype(mmnp),
        })
    return in_maps


def kernel(x, w_qkv, w_out, b_out):
    from concourse.bass_utils import run_bass_kernel_spmd

    x = np.asarray(x, dtype=np.float32)
    w_qkv = np.asarray(w_qkv, dtype=np.float32)
    w_out = np.asarray(w_out, dtype=np.float32)
    b_out = np.asarray(b_out, dtype=np.float32)

    nc = get_nc()
    in_maps = make_core_inputs(x, w_qkv, w_out)
    res = run_bass_kernel_spmd(nc, in_maps, list(range(NCORES))).results

    out = np.empty((B, SEQ, DIM), dtype=np.float32)
    for b in range(B):
        out[b] = res[2 * b]["y"] + res[2 * b + 1]["y"] + b_out
    return out


# revision 35
# speedup vs baseline: 1.6420x; 1.3403x over previous
"""Multi-head self-attention forward on 8 Trainium2 NeuronCores.

Problem: x[4,2048,512] -> qkv proj (w_qkv [512,1536]) -> 8-head attention
(head_dim 64) -> out proj (w_out [512,512] + b_out) -> y[4,2048,512].

Sharding: 8 shards = (batch b in 0..3) x (head-group hg in 0..1, 4 heads each).
Core c handles b=c//2, hg=c%2. Each core computes, for its batch and its 4
heads: qkv projection (only its heads' columns), attention, and the partial
output projection restricted to its heads' rows of w_out. Host sums the two
half-projections per batch and adds the bias.

On-device layout (all "T" tensors keep the contraction dim on partitions):
  xT   [512, 2048]   x[b] transposed (host-side transpose)
  qkT  4 tiles [128, 2048]: Q01, K01, Q23, K23 (2 heads stacked per tile:
       head A on partitions 0:64, head B on 64:128)
  v_aug 16 seq-tiles [128, 4*65]: per head 64 v columns + a ones column
       (the ones column makes the oT matmul also produce the softmax
       denominator as row 64 of its output)
  sT   [k, q] scores transposed -> exp (no max subtraction: |s|~N(0,1), safe
       in fp32) -> pT
  oT   v_aug.T @ pT = [65, q]: rows 0:64 unnormalized head output (d on
       partitions), row 64 = softmax denominator
  yproj y[q,c] per head = oT_head.T @ w2_head, scaled per-partition (q) by
       1/denom via tensor_scalar, summed over the 4 heads on DVE.
"""

import numpy as np

import concourse.bass as bass
import concourse.mybir as mybir
import concourse.tile as tile
from concourse import bacc
from concourse.masks import make_identity

DIM = 512
NHEADS = 8
HD = 64
B = 4
SEQ = 2048
SCALE = HD ** -0.5

NCORES = 8
HPC = 4          # heads per core
QCH = 512        # q chunk (moving free dim)
NQC = SEQ // QCH # 4 q-chunks
KCH = 128        # k chunk (psum partition dim)
NKC = SEQ // KCH # 16 k-chunks
CCH = 128        # contraction chunk for projections
NCC = DIM // CCH # 4

F32 = mybir.dt.float32
F32R = mybir.dt.float32r

BF16 = mybir.dt.bfloat16
# matmul input dtype. bf16: 1 cycle/row guaranteed, FWL weight loads, and
# roughly half the PE power of f32r (the f32r version HAM-throttled to
# 1.2GHz for 80% of the run). f32r kept as a fallback for precision.
MMDT = BF16


ROW_TILE_S = True


def _mm(ap):
    return ap


def _emit_o(nc, oA, oB, vaug_t, pt_pair, i, p, start, stop):
    """Accumulate the two kc chunks of pair-iteration i into oA/oB."""
    pA, pB = pt_pair
    QCH = 512
    for hh, (odst, psrc) in enumerate(((oA, pA), (oB, pB))):
        for half in range(2):
            kc = 2 * i + half
            nc.tensor.matmul(
                odst[:],
                _mm(vaug_t(kc)[:, 2 * p + hh, :]),
                _mm(psrc[:, half * QCH:(half + 1) * QCH]),
                start=(start and half == 0), stop=(stop and half == 1),
                skip_group_check=True,
            )


def build_nc():
    nc = bacc.Bacc()

    xT_d = nc.dram_tensor("xt", [DIM, SEQ], MMDT, kind="ExternalInput")
    wperm_d = nc.dram_tensor("wperm", [DIM, 4 * 128], MMDT, kind="ExternalInput")
    wv_d = nc.dram_tensor("wv", [DIM, HPC * HD], MMDT, kind="ExternalInput")
    w2_d = nc.dram_tensor("w2", [HPC * HD, DIM], MMDT, kind="ExternalInput")
    y_d = nc.dram_tensor("y", [SEQ, DIM], F32, kind="ExternalOutput")

    with tile.TileContext(nc) as tc:
        with (
            tc.tile_pool(name="const", bufs=1) as cpool,
            tc.tile_pool(name="big", bufs=1) as bigpool,
            tc.tile_pool(name="pt", bufs=3) as ptpool,
            tc.tile_pool(name="yacc", bufs=1) as yaccpool,
            tc.tile_pool(name="tmp", bufs=3) as tmppool,
            tc.tile_pool(name="small", bufs=2) as smallpool,
            tc.tile_pool(name="ps", bufs=1, space="PSUM") as ps,
        ):
            # ---- constants / inputs to SBUF ----
            # DMA lands fp32 in a staging tile; a DVE copy rounds into the
            # f32r tile the matmuls read (BIR f32r-rounding requirement).
            xTs = [cpool.tile([128, SEQ], MMDT, tag=f"xT{c}", name=f"xT{c}")
                   for c in range(NCC)]
            wps = [cpool.tile([128, 512], MMDT, tag=f"wp{c}", name=f"wp{c}")
                   for c in range(NCC)]
            wvs = [cpool.tile([128, HPC * HD], MMDT, tag=f"wv{c}", name=f"wv{c}")
                   for c in range(NCC)]
            w2s = [cpool.tile([128, DIM], MMDT, tag=f"w2{p}", name=f"w2{p}")
                   for p in range(2)]
            ident = cpool.tile([2, 2], F32, tag="ident")
            ones4 = cpool.tile([128, HPC], F32, tag="ones4")
            nc.gpsimd.memset(ones4[:], 1.0)
            ones1 = cpool.tile([1, 1], F32, tag="ones1")
            nc.gpsimd.memset(ones1[:], 1.0)
            # preload the exp ACT table set early so the first real exp in
            # the attention phase doesn't stall the pipeline ~2.7us (the
            # PE gap there is what re-throttles HAM to K=4/8)
            dummy = cpool.tile([1, 1], F32, tag="dummy")
            nc.scalar.activation(dummy[:], ones1[:],
                                 mybir.ActivationFunctionType.Exp)

            for c in range(NCC):
                nc.sync.dma_start(xTs[c][:], xT_d[c * 128:(c + 1) * 128, :])
                nc.sync.dma_start(wps[c][:], wperm_d[c * 128:(c + 1) * 128, :])
                nc.sync.dma_start(wvs[c][:], wv_d[c * 128:(c + 1) * 128, :])
            for p in range(2):
                nc.sync.dma_start(w2s[p][:], w2_d[p * 128:(p + 1) * 128, :])
            make_identity(nc, ident[:])

            def xT_c(c):
                return xTs[c]

            # ---- persistent intermediates ----
            qkT = bigpool.tile([128, 4 * SEQ], MMDT, tag="qkT")  # Q01 K01 Q23 K23
            vaug = bigpool.tile([128, NKC * HPC * 65], MMDT, tag="vaug")
            oT = bigpool.tile([128, 2 * SEQ], MMDT, tag="oT")    # pair-packed

            yacc = yaccpool.tile([128, SEQ // 128 * DIM], F32, tag="yacc")

            def qkT_blk(m):
                return qkT[:, m * SEQ:(m + 1) * SEQ]

            def vaug_t(kc):
                # [128, HPC, 65] view of seq-tile kc
                return vaug[:, kc * HPC * 65:(kc + 1) * HPC * 65].rearrange(
                    "p (h e) -> p h e", e=65
                )

            # ---- phase 1a: qkT = wperm.T @ xT ----
            for m in range(4):
                for s in range(SEQ // 1024):
                    pp = ps.tile([128, 1024], F32, tag="sA", bufs=1, name="pp")
                    for c in range(NCC):
                        for half in range(2):
                            nc.tensor.matmul(
                                pp[:, half * 512:(half + 1) * 512],
                                _mm(wps[c][:, m * 128:(m + 1) * 128]),
                                _mm(xT_c(c)[:, s * 1024 + half * 512:
                                            s * 1024 + (half + 1) * 512]),
                                start=(c == 0),
                                stop=(c == NCC - 1),
                                skip_group_check=True,
                            )
                    nc.vector.tensor_copy(
                        qkT_blk(m)[:, s * 1024:(s + 1) * 1024], pp[:]
                    )

            # ---- phase 1b: v = x @ wv (natural layout, + ones column) ----
            for st in range(NKC):
                pv = ps.tile([128, HPC * HD], F32, tag="sB", bufs=1, name="pv")
                for c in range(NCC):
                    nc.tensor.matmul(
                        pv[:],
                        _mm(xT_c(c)[:, st * 128:(st + 1) * 128]),
                        _mm(wvs[c][:]),
                        start=(c == 0),
                        stop=(c == NCC - 1),
                        skip_group_check=True,
                    )
                vt = vaug_t(st)
                nc.vector.tensor_copy(
                    vt[:, :, 0:64], pv[:].rearrange("p (h d) -> p h d", d=HD)
                )
                nc.vector.tensor_copy(vt[:, :, 64:65],
                                      ones4[:].rearrange("p (h o) -> p h o", o=1))

            # ---- phase 2: attention + out-proj, per head-pair ----
            # kc chunks processed in pairs: one s psum tile [128, 1024] holds
            # scores for kc and kc+1 side by side (same 128 k-partitions map
            # to different k-blocks per column half; exp is elementwise so it
            # doesn't care), halving ACT instruction count. The PE stream is
            # software-pipelined: s(i+1) is emitted before o(i) so the PE
            # never head-of-line blocks on the exp of iteration i.
            NPAIR = NKC // 2
            for p in range(2):
                Q = qkT_blk(2 * p)
                K = qkT_blk(2 * p + 1)
                for qc in range(NQC):
                    oA = ps.tile([65, QCH], F32, tag="oA", bufs=1, name="oA")
                    oB = ps.tile([65, QCH], F32, tag="oB", bufs=1, name="oB")
                    prev = None
                    for i in range(NPAIR):
                        sA = ps.tile([128, 2 * QCH], F32, tag="sA", bufs=1,
                                     name="sA")
                        sB = ps.tile([128, 2 * QCH], F32, tag="sB", bufs=1,
                                     name="sB")
                        for hh, stile in ((0, sA), (1, sB)):
                            tp = (64 * hh, 0) if ROW_TILE_S else None
                            for half in range(2):
                                kc = 2 * i + half
                                nc.tensor.matmul(
                                    stile[:, half * QCH:(half + 1) * QCH],
                                    _mm(K[64 * hh:64 * hh + 64,
                                          kc * 128:(kc + 1) * 128]),
                                    _mm(Q[64 * hh:64 * hh + 64,
                                          qc * QCH:(qc + 1) * QCH]),
                                    start=True, stop=True,
                                    skip_group_check=True,
                                    tile_position=tp,
                                )
                        if prev is not None:
                            _emit_o(nc, oA, oB, vaug_t, prev, i - 1, p,
                                    start=(i == 1), stop=(i == NPAIR - 1))
                        pA = ptpool.tile([128, 2 * QCH], MMDT, tag="pA")
                        pB = ptpool.tile([128, 2 * QCH], MMDT, tag="pB")
                        nc.scalar.activation(
                            pA[:], sA[:], mybir.ActivationFunctionType.Exp,
                            scale=SCALE,
                        )
                        nc.scalar.activation(
                            pB[:], sB[:], mybir.ActivationFunctionType.Exp,
                            scale=SCALE,
                        )
                        prev = (pA, pB)
                    _emit_o(nc, oA, oB, vaug_t, prev, NPAIR - 1, p,
                            start=(NPAIR == 1), stop=True)

                    # evacuate oT + denominators
                    nc.vector.tensor_copy(oT[0:64, p * SEQ + qc * QCH:
                                             p * SEQ + (qc + 1) * QCH],
                                          oA[0:64, :])
                    nc.vector.tensor_copy(oT[64:128, p * SEQ + qc * QCH:
                                             p * SEQ + (qc + 1) * QCH],
                                          oB[0:64, :])
                    denA = smallpool.tile([1, QCH], F32, tag="denA")
                    denB = smallpool.tile([1, QCH], F32, tag="denB")
                    nc.vector.tensor_copy(denA[:], oA[64:65, :])
                    nc.vector.tensor_copy(denB[:], oB[64:65, :])

                    # denominators -> per-partition layout via K=1 matmuls
                    # (is_transpose matmuls crash the HW here), then one
                    # 128-lane reciprocal
                    rt_ps = ps.tile([128, 2 * (QCH // 128)], F32, tag="y",
                                    bufs=2, name="rt_ps")
                    for j in range(QCH // 128):
                        for h, rv in enumerate((denA, denB)):
                            nc.tensor.matmul(
                                rt_ps[:, 2 * j + h:2 * j + h + 1],
                                rv[:, j * 128:(j + 1) * 128],
                                ones1[:],
                                start=True, stop=True, skip_group_check=True,
                            )
                    dt_sb = smallpool.tile([128, 2 * (QCH // 128)], F32,
                                           tag="dt_sb")
                    nc.vector.tensor_copy(dt_sb[:], rt_ps[:])
                    rt = smallpool.tile([128, 2 * (QCH // 128)], F32,
                                        tag="rt_sb")
                    nc.vector.reciprocal(rt[:], dt_sb[:])

                    # out-projection + normalization for this q-chunk
                    for j in range(QCH // 128):
                        qt = qc * (QCH // 128) + j
                        yA = ps.tile([128, DIM], F32, tag="y", bufs=2,
                                     name="yA")
                        yB = ps.tile([128, DIM], F32, tag="y", bufs=2,
                                     name="yB")
                        oTp = oT[:, p * SEQ:(p + 1) * SEQ]
                        nc.tensor.matmul(
                            yA[:],
                            _mm(oTp[0:64, qt * 128:(qt + 1) * 128]),
                            _mm(w2s[p][0:64, :]),
                            start=True, stop=True, skip_group_check=True,
                        )
                        nc.tensor.matmul(
                            yB[:],
                            _mm(oTp[64:128, qt * 128:(qt + 1) * 128]),
                            _mm(w2s[p][64:128, :]),
                            start=True, stop=True, skip_group_check=True,
                        )
                        ya = yacc[:, qt * DIM:(qt + 1) * DIM]
                        if p == 0:
                            nc.vector.tensor_scalar_mul(
                                ya, yA[:], rt[:, 2 * j:2 * j + 1])
                            t1 = tmppool.tile([128, DIM], F32, tag="t1")
                            nc.vector.tensor_scalar_mul(
                                t1[:], yB[:], rt[:, 2 * j + 1:2 * j + 2])
                            nc.vector.tensor_add(ya, ya, t1[:])
                        else:
                            t1 = tmppool.tile([128, DIM], F32, tag="t1")
                            nc.vector.tensor_scalar_mul(
                                t1[:], yA[:], rt[:, 2 * j:2 * j + 1])
                            nc.vector.tensor_add(ya, ya, t1[:])
                            t2 = tmppool.tile([128, DIM], F32, tag="t2")
                            nc.vector.tensor_scalar_mul(
                                t2[:], yB[:], rt[:, 2 * j + 1:2 * j + 2])
                            nc.vector.tensor_add(ya, ya, t2[:])
                            nc.sync.dma_start(
                                y_d[qt * 128:(qt + 1) * 128, :], ya)

    nc.finalize()
    return nc


_NC_CACHE = {}


def get_nc():
    if "nc" not in _NC_CACHE:
        _NC_CACHE["nc"] = build_nc()
    return _NC_CACHE["nc"]


def make_core_inputs(x, w_qkv, w_out):
    """Per-core input dicts (host-side sharding)."""
    in_maps = []
    for c in range(NCORES):
        b, hg = c // 2, c % 2
        heads = [hg * HPC + i for i in range(HPC)]
        qcols = [w_qkv[:, h * HD:(h + 1) * HD] for h in heads]
        kcols = [w_qkv[:, DIM + h * HD:DIM + (h + 1) * HD] for h in heads]
        vcols = [w_qkv[:, 2 * DIM + h * HD:2 * DIM + (h + 1) * HD] for h in heads]
        wperm = np.concatenate(
            [qcols[0], qcols[1], kcols[0], kcols[1],
             qcols[2], qcols[3], kcols[2], kcols[3]], axis=1)
        wv = np.concatenate(vcols, axis=1)
        w2 = w_out[hg * HPC * HD:(hg + 1) * HPC * HD, :]
        import ml_dtypes
        mmnp = (ml_dtypes.bfloat16 if MMDT == mybir.dt.bfloat16
                else np.float32)
        in_maps.append({
            "xt": np.ascontiguousarray(x[b].T).astype(mmnp),
            "wperm": np.ascontiguousarray(wperm).astype(mmnp),
            "wv": np.ascontiguousarray(wv).astype(mmnp),
            "w2": np.ascontiguousarray(w2).astype(mmnp),
        })
    return in_maps


def kernel(x, w_qkv, w_out, b_out):
    from concourse.bass_utils import run_bass_kernel_spmd

    x = np.asarray(x, dtype=np.float32)
    w_qkv = np.asarray(w_qkv, dtype=np.float32)
    w_out = np.asarray(w_out, dtype=np.float32)
    b_out = np.asarray(b_out, dtype=np.float32)

    nc = get_nc()
    in_maps = make_core_inputs(x, w_qkv, w_out)
    res = run_bass_kernel_spmd(nc, in_maps, list(range(NCORES))).results

    out = np.empty((B, SEQ, DIM), dtype=np.float32)
    for b in range(B):
        out[b] = res[2 * b]["y"] + res[2 * b + 1]["y"] + b_out
    return out


# revision 36
# speedup vs baseline: 1.8048x; 1.0991x over previous
"""Multi-head self-attention forward on 8 Trainium2 NeuronCores.

Problem: x[4,2048,512] -> qkv proj (w_qkv [512,1536]) -> 8-head attention
(head_dim 64) -> out proj (w_out [512,512] + b_out) -> y[4,2048,512].

Sharding: 8 shards = (batch b in 0..3) x (head-group hg in 0..1, 4 heads each).
Core c handles b=c//2, hg=c%2. Each core computes, for its batch and its 4
heads: qkv projection (only its heads' columns), attention, and the partial
output projection restricted to its heads' rows of w_out. Host sums the two
half-projections per batch and adds the bias.

On-device layout (all "T" tensors keep the contraction dim on partitions):
  xT   [512, 2048]   x[b] transposed (host-side transpose)
  qkT  4 tiles [128, 2048]: Q01, K01, Q23, K23 (2 heads stacked per tile:
       head A on partitions 0:64, head B on 64:128)
  v_aug 16 seq-tiles [128, 4*65]: per head 64 v columns + a ones column
       (the ones column makes the oT matmul also produce the softmax
       denominator as row 64 of its output)
  sT   [k, q] scores transposed -> exp (no max subtraction: |s|~N(0,1), safe
       in fp32) -> pT
  oT   v_aug.T @ pT = [65, q]: rows 0:64 unnormalized head output (d on
       partitions), row 64 = softmax denominator
  yproj y[q,c] per head = oT_head.T @ w2_head, scaled per-partition (q) by
       1/denom via tensor_scalar, summed over the 4 heads on DVE.
"""

import numpy as np

import concourse.bass as bass
import concourse.mybir as mybir
import concourse.tile as tile
from concourse import bacc
from concourse.masks import make_identity

DIM = 512
NHEADS = 8
HD = 64
B = 4
SEQ = 2048
SCALE = HD ** -0.5

NCORES = 8
HPC = 4          # heads per core
QCH = 512        # q chunk (moving free dim)
NQC = SEQ // QCH # 4 q-chunks
KCH = 128        # k chunk (psum partition dim)
NKC = SEQ // KCH # 16 k-chunks
CCH = 128        # contraction chunk for projections
NCC = DIM // CCH # 4

F32 = mybir.dt.float32
F32R = mybir.dt.float32r

BF16 = mybir.dt.bfloat16
# matmul input dtype. bf16: 1 cycle/row guaranteed, FWL weight loads, and
# roughly half the PE power of f32r (the f32r version HAM-throttled to
# 1.2GHz for 80% of the run). f32r kept as a fallback for precision.
MMDT = BF16


ROW_TILE_S = False


def _mm(ap):
    return ap


def _emit_o(nc, oA, oB, vaug_t, pt_pair, i, p, start, stop):
    """Accumulate the two kc chunks of pair-iteration i into oA/oB."""
    pA, pB = pt_pair
    QCH = 512
    for hh, (odst, psrc) in enumerate(((oA, pA), (oB, pB))):
        for half in range(2):
            kc = 2 * i + half
            nc.tensor.matmul(
                odst[:],
                _mm(vaug_t(kc)[:, 2 * p + hh, :]),
                _mm(psrc[:, half * QCH:(half + 1) * QCH]),
                start=(start and half == 0), stop=(stop and half == 1),
                skip_group_check=True,
            )


def build_nc():
    nc = bacc.Bacc()

    xT_d = nc.dram_tensor("xt", [DIM, SEQ], MMDT, kind="ExternalInput")
    wperm_d = nc.dram_tensor("wperm", [DIM, 4 * 128], MMDT, kind="ExternalInput")
    wv_d = nc.dram_tensor("wv", [DIM, HPC * HD], MMDT, kind="ExternalInput")
    w2_d = nc.dram_tensor("w2", [HPC * HD, DIM], MMDT, kind="ExternalInput")
    y_d = nc.dram_tensor("y", [SEQ, DIM], F32, kind="ExternalOutput")

    with tile.TileContext(nc) as tc:
        with (
            tc.tile_pool(name="const", bufs=1) as cpool,
            tc.tile_pool(name="big", bufs=1) as bigpool,
            tc.tile_pool(name="pt", bufs=3) as ptpool,
            tc.tile_pool(name="yacc", bufs=1) as yaccpool,
            tc.tile_pool(name="tmp", bufs=3) as tmppool,
            tc.tile_pool(name="small", bufs=2) as smallpool,
            tc.tile_pool(name="ps", bufs=1, space="PSUM") as ps,
        ):
            # ---- constants / inputs to SBUF ----
            # DMA lands fp32 in a staging tile; a DVE copy rounds into the
            # f32r tile the matmuls read (BIR f32r-rounding requirement).
            xTs = [cpool.tile([128, SEQ], MMDT, tag=f"xT{c}", name=f"xT{c}")
                   for c in range(NCC)]
            wps = [cpool.tile([128, 512], MMDT, tag=f"wp{c}", name=f"wp{c}")
                   for c in range(NCC)]
            wvs = [cpool.tile([128, HPC * HD], MMDT, tag=f"wv{c}", name=f"wv{c}")
                   for c in range(NCC)]
            w2s = [cpool.tile([128, DIM], MMDT, tag=f"w2{p}", name=f"w2{p}")
                   for p in range(2)]
            ident = cpool.tile([2, 2], F32, tag="ident")
            ones4 = cpool.tile([128, HPC], F32, tag="ones4")
            nc.gpsimd.memset(ones4[:], 1.0)
            ones1 = cpool.tile([1, 1], F32, tag="ones1")
            nc.gpsimd.memset(ones1[:], 1.0)
            # preload the exp ACT table set early so the first real exp in
            # the attention phase doesn't stall the pipeline ~2.7us (the
            # PE gap there is what re-throttles HAM to K=4/8)
            dummy = cpool.tile([1, 1], F32, tag="dummy")
            nc.scalar.activation(dummy[:], ones1[:],
                                 mybir.ActivationFunctionType.Exp)

            nc.sync.dma_start(wps[0][:], wperm_d[0:128, :])
            for c in range(NCC):
                # first 1024 cols of xT chunk c, then the rest: the first
                # qkT matmuls only need the leading seq block
                nc.sync.dma_start(xTs[c][:, 0:1024],
                                  xT_d[c * 128:(c + 1) * 128, 0:1024])
            for c in range(1, NCC):
                nc.sync.dma_start(wps[c][:], wperm_d[c * 128:(c + 1) * 128, :])
            for c in range(NCC):
                nc.sync.dma_start(xTs[c][:, 1024:SEQ],
                                  xT_d[c * 128:(c + 1) * 128, 1024:SEQ])
                nc.sync.dma_start(wvs[c][:], wv_d[c * 128:(c + 1) * 128, :])
            for p in range(2):
                nc.sync.dma_start(w2s[p][:], w2_d[p * 128:(p + 1) * 128, :])
            make_identity(nc, ident[:])

            def xT_c(c):
                return xTs[c]

            # ---- persistent intermediates ----
            qkT = bigpool.tile([128, 4 * SEQ], MMDT, tag="qkT")  # Q01 K01 Q23 K23
            vaug = bigpool.tile([128, NKC * HPC * 65], MMDT, tag="vaug")
            oT = bigpool.tile([128, 2 * SEQ], MMDT, tag="oT")    # pair-packed

            yacc = yaccpool.tile([128, SEQ // 128 * DIM], F32, tag="yacc")

            def qkT_blk(m):
                return qkT[:, m * SEQ:(m + 1) * SEQ]

            def vaug_t(kc):
                # [128, HPC, 65] view of seq-tile kc
                return vaug[:, kc * HPC * 65:(kc + 1) * HPC * 65].rearrange(
                    "p (h e) -> p h e", e=65
                )

            # ---- phase 1a: qkT = wperm.T @ xT ----
            for m in range(4):
                for s in range(SEQ // 1024):
                    pp = ps.tile([128, 1024], F32, tag="sA", bufs=1, name="pp")
                    for c in range(NCC):
                        for half in range(2):
                            nc.tensor.matmul(
                                pp[:, half * 512:(half + 1) * 512],
                                _mm(wps[c][:, m * 128:(m + 1) * 128]),
                                _mm(xT_c(c)[:, s * 1024 + half * 512:
                                            s * 1024 + (half + 1) * 512]),
                                start=(c == 0),
                                stop=(c == NCC - 1),
                                skip_group_check=True,
                            )
                    nc.vector.tensor_copy(
                        qkT_blk(m)[:, s * 1024:(s + 1) * 1024], pp[:]
                    )

            # ---- phase 1b: v = x @ wv (natural layout, + ones column) ----
            for st in range(NKC):
                pv = ps.tile([128, HPC * HD], F32, tag="sB", bufs=1, name="pv")
                for c in range(NCC):
                    nc.tensor.matmul(
                        pv[:],
                        _mm(xT_c(c)[:, st * 128:(st + 1) * 128]),
                        _mm(wvs[c][:]),
                        start=(c == 0),
                        stop=(c == NCC - 1),
                        skip_group_check=True,
                    )
                vt = vaug_t(st)
                nc.vector.tensor_copy(
                    vt[:, :, 0:64], pv[:].rearrange("p (h d) -> p h d", d=HD)
                )
                nc.vector.tensor_copy(vt[:, :, 64:65],
                                      ones4[:].rearrange("p (h o) -> p h o", o=1))

            # ---- phase 2: attention + out-proj, per head-pair ----
            # kc chunks processed in pairs: one s psum tile [128, 1024] holds
            # scores for kc and kc+1 side by side (same 128 k-partitions map
            # to different k-blocks per column half; exp is elementwise so it
            # doesn't care), halving ACT instruction count. The PE stream is
            # software-pipelined: s(i+1) is emitted before o(i) so the PE
            # never head-of-line blocks on the exp of iteration i.
            NPAIR = NKC // 2
            for p in range(2):
                Q = qkT_blk(2 * p)
                K = qkT_blk(2 * p + 1)
                for qc in range(NQC):
                    oA = ps.tile([65, QCH], F32, tag="oA", bufs=1, name="oA")
                    oB = ps.tile([65, QCH], F32, tag="oB", bufs=1, name="oB")
                    prev = None
                    for i in range(NPAIR):
                        sA = ps.tile([128, 2 * QCH], F32, tag="sA", bufs=1,
                                     name="sA")
                        sB = ps.tile([128, 2 * QCH], F32, tag="sB", bufs=1,
                                     name="sB")
                        for hh, stile in ((0, sA), (1, sB)):
                            tp = (64 * hh, 0) if ROW_TILE_S else None
                            for half in range(2):
                                kc = 2 * i + half
                                nc.tensor.matmul(
                                    stile[:, half * QCH:(half + 1) * QCH],
                                    _mm(K[64 * hh:64 * hh + 64,
                                          kc * 128:(kc + 1) * 128]),
                                    _mm(Q[64 * hh:64 * hh + 64,
                                          qc * QCH:(qc + 1) * QCH]),
                                    start=True, stop=True,
                                    skip_group_check=True,
                                    tile_position=tp,
                                )
                        if prev is not None:
                            _emit_o(nc, oA, oB, vaug_t, prev, i - 1, p,
                                    start=(i == 1), stop=(i == NPAIR - 1))
                        pA = ptpool.tile([128, 2 * QCH], MMDT, tag="pA")
                        pB = ptpool.tile([128, 2 * QCH], MMDT, tag="pB")
                        nc.scalar.activation(
                            pA[:], sA[:], mybir.ActivationFunctionType.Exp,
                            scale=SCALE,
                        )
                        nc.scalar.activation(
                            pB[:], sB[:], mybir.ActivationFunctionType.Exp,
                            scale=SCALE,
                        )
                        prev = (pA, pB)
                    _emit_o(nc, oA, oB, vaug_t, prev, NPAIR - 1, p,
                            start=(NPAIR == 1), stop=True)

                    # evacuate oT + denominators
                    nc.vector.tensor_copy(oT[0:64, p * SEQ + qc * QCH:
                                             p * SEQ + (qc + 1) * QCH],
                                          oA[0:64, :])
                    nc.vector.tensor_copy(oT[64:128, p * SEQ + qc * QCH:
                                             p * SEQ + (qc + 1) * QCH],
                                          oB[0:64, :])
                    denA = smallpool.tile([1, QCH], F32, tag="denA")
                    denB = smallpool.tile([1, QCH], F32, tag="denB")
                    nc.vector.tensor_copy(denA[:], oA[64:65, :])
                    nc.vector.tensor_copy(denB[:], oB[64:65, :])

                    # denominators -> per-partition layout via K=1 matmuls
                    # (is_transpose matmuls crash the HW here), then one
                    # 128-lane reciprocal
                    rt_ps = ps.tile([128, 2 * (QCH // 128)], F32, tag="y",
                                    bufs=2, name="rt_ps")
                    for j in range(QCH // 128):
                        for h, rv in enumerate((denA, denB)):
                            nc.tensor.matmul(
                                rt_ps[:, 2 * j + h:2 * j + h + 1],
                                rv[:, j * 128:(j + 1) * 128],
                                ones1[:],
                                start=True, stop=True, skip_group_check=True,
                            )
                    dt_sb = smallpool.tile([128, 2 * (QCH // 128)], F32,
                                           tag="dt_sb")
                    nc.vector.tensor_copy(dt_sb[:], rt_ps[:])
                    rt = smallpool.tile([128, 2 * (QCH // 128)], F32,
                                        tag="rt_sb")
                    nc.vector.reciprocal(rt[:], dt_sb[:])

                    # out-projection + normalization for this q-chunk
                    for j in range(QCH // 128):
                        qt = qc * (QCH // 128) + j
                        yA = ps.tile([128, DIM], F32, tag="y", bufs=2,
                                     name="yA")
                        yB = ps.tile([128, DIM], F32, tag="y", bufs=2,
                                     name="yB")
                        oTp = oT[:, p * SEQ:(p + 1) * SEQ]
                        nc.tensor.matmul(
                            yA[:],
                            _mm(oTp[0:64, qt * 128:(qt + 1) * 128]),
                            _mm(w2s[p][0:64, :]),
                            start=True, stop=True, skip_group_check=True,
                        )
                        nc.tensor.matmul(
                            yB[:],
                            _mm(oTp[64:128, qt * 128:(qt + 1) * 128]),
                            _mm(w2s[p][64:128, :]),
                            start=True, stop=True, skip_group_check=True,
                        )
                        ya = yacc[:, qt * DIM:(qt + 1) * DIM]
                        if p == 0:
                            nc.vector.tensor_scalar_mul(
                                ya, yA[:], rt[:, 2 * j:2 * j + 1])
                            t1 = tmppool.tile([128, DIM], F32, tag="t1")
                            nc.vector.tensor_scalar_mul(
                                t1[:], yB[:], rt[:, 2 * j + 1:2 * j + 2])
                            nc.gpsimd.tensor_add(ya, ya, t1[:])
                        else:
                            t1 = tmppool.tile([128, DIM], F32, tag="t1")
                            nc.vector.tensor_scalar_mul(
                                t1[:], yA[:], rt[:, 2 * j:2 * j + 1])
                            nc.gpsimd.tensor_add(ya, ya, t1[:])
                            t2 = tmppool.tile([128, DIM], F32, tag="t2")
                            nc.vector.tensor_scalar_mul(
                                t2[:], yB[:], rt[:, 2 * j + 1:2 * j + 2])
                            nc.gpsimd.tensor_add(ya, ya, t2[:])
                            nc.sync.dma_start(
                                y_d[qt * 128:(qt + 1) * 128, :], ya)

    nc.finalize()
    return nc


_NC_CACHE = {}


def get_nc():
    if "nc" not in _NC_CACHE:
        _NC_CACHE["nc"] = build_nc()
    return _NC_CACHE["nc"]


def make_core_inputs(x, w_qkv, w_out):
    """Per-core input dicts (host-side sharding)."""
    in_maps = []
    for c in range(NCORES):
        b, hg = c // 2, c % 2
        heads = [hg * HPC + i for i in range(HPC)]
        qcols = [w_qkv[:, h * HD:(h + 1) * HD] for h in heads]
        kcols = [w_qkv[:, DIM + h * HD:DIM + (h + 1) * HD] for h in heads]
        vcols = [w_qkv[:, 2 * DIM + h * HD:2 * DIM + (h + 1) * HD] for h in heads]
        wperm = np.concatenate(
            [qcols[0], qcols[1], kcols[0], kcols[1],
             qcols[2], qcols[3], kcols[2], kcols[3]], axis=1)
        wv = np.concatenate(vcols, axis=1)
        w2 = w_out[hg * HPC * HD:(hg + 1) * HPC * HD, :]
        import ml_dtypes
        mmnp = (ml_dtypes.bfloat16 if MMDT == mybir.dt.bfloat16
                else np.float32)
        in_maps.append({
            "xt": np.ascontiguousarray(x[b].T).astype(mmnp),
            "wperm": np.ascontiguousarray(wperm).astype(mmnp),
            "wv": np.ascontiguousarray(wv).astype(mmnp),
            "w2": np.ascontiguousarray(w2).astype(mmnp),
        })
    return in_maps


def kernel(x, w_qkv, w_out, b_out):
    from concourse.bass_utils import run_bass_kernel_spmd

    x = np.asarray(x, dtype=np.float32)
    w_qkv = np.asarray(w_qkv, dtype=np.float32)
    w_out = np.asarray(w_out, dtype=np.float32)
    b_out = np.asarray(b_out, dtype=np.float32)

    nc = get_nc()
    in_maps = make_core_inputs(x, w_qkv, w_out)
    res = run_bass_kernel_spmd(nc, in_maps, list(range(NCORES))).results

    out = np.empty((B, SEQ, DIM), dtype=np.float32)
    for b in range(B):
        out[b] = res[2 * b]["y"] + res[2 * b + 1]["y"] + b_out
    return out
